# revision 12
# baseline (speedup 1.0000x reference)
"""FreeConv2D (locally-connected conv2d + bias) Trainium2 Bass kernel.

out[b,oh,ow,u] = sum_{i,j,c} w[oh,ow,u,i,j,c] * x[b, oh*2+i, ow*2+j, c] + bias[oh,ow,u]

Shapes: x [64,64,64,64], w [30,30,64,5,5,64], b [30,30,64] -> out [64,30,30,64].

Strategy (8 NeuronCores):
  - Shard output rows OH over cores: 4 rows/core (padded 30->32; last 2 dummy).
  - The kernel is DMA-bound (~330 GB/s/core aggregate): w dominates traffic,
    so the w stream is stored as float8_e3m4 * 32 (half the bytes of fp16;
    measured rel err ~1.1e-2 vs the 2e-2 gate) and the output as bf16. The
    matmul mixes lhsT fp16 (x) with rhs fp8e3 (w) — allowed on TRN2.
  - PSUM tiles are not memset: the first matmul into each (oh, role) slot
    uses start=True (even-r blocks are split so the fresh-oh part is its own
    matmul), which keeps the DVE free for drains.
  - Host pre-packs (numpy, not counted in HW time):
      * x    -> per-core fp16 tile [128, 11*32*64]: partition p = dj*64+c for
               column pair (2*mp, 2*mp+1), free = (r, mp, b).
      * w    -> per-core fp8e3 stream [128, TOT] (values * 32): matmul rhs
               blocks in execution order (column-pair taps j in {0,1} / {2,3}
               as K=128 blocks; j=4 taps as K=64 vertically-paired blocks).
      * bias -> per-core fp32 [64, 30*4*64] * 32 replicated over batch
               partitions; host gather divides the 32 back out.
  - Device: 32-phase sweep over column pairs mp. Phase mp:
      * DMA the phase's w blocks (~1 MB).
      * psum phase tile pt[mp] [64, 512] = accum slots (oh_l, role) where
        role 0 = j01-half of loc (oh, mp), role 1 = j23-half of loc (oh, mp-1).
      * matmuls: lhsT = resident x tile [128, 64(b)] (stationary),
        rhs = w blocks [128, N<=512] (moving), accumulate with start=False
        (tiles pre-zeroed by DVE memset; psum has_written semantics make this
        correct whether the first PE write accumulates or overwrites).
      * j=4 taps (K=64) of loc (oh, mp-2) also land in pt[mp-2] role-0 slots.
      * drain loc (.., ow=mp-2): out = pt[ow].role0 + bias + pt[ow+1].role1
        via two DVE tensor_adds into an SBUF staging buffer.
  - One final DMA of staging -> DRAM out [64, 30(ow), 4(oh_l), 64] per core;
    host gathers/transposes/trims to [64, 30, 30, 64].
"""

import os
import sys

import numpy as np

_TRN_REPO = "/opt/trn_rl_repo"
if _TRN_REPO not in sys.path:
    sys.path.insert(0, _TRN_REPO)

# The kernel needs the axon/neuron jax backend; a JAX_PLATFORMS=cpu pin (used
# for reference computation) would hide the NeuronCores. Only effective if jax
# has not been initialized yet in this process.
if "jax" not in sys.modules and "axon" not in os.environ.get("JAX_PLATFORMS", "axon"):
    os.environ.pop("JAX_PLATFORMS", None)

# ---------------- problem constants (hardcoded) ----------------
B, H, W, C = 64, 64, 64, 64
U, K, S = 64, 5, 2
OH = OW = 30
NCORES = 8
NO = 4                      # oh rows per core (padded: 8*4 = 32 >= 30)
OHP = NCORES * NO           # 32
NR = 2 * (NO - 1) + K       # 11 input rows per core
NMP = 32                    # column-pair tiles mp=0..31; also phase count
NT4 = OW // 2               # 15 j4 ow-pairs
HP = 2 * (OHP - 1) + K      # 67 padded input rows overall


def _oh_span(r):
    """Valid local oh range for local input row r: i = r - 2*oh in [0, K-1]."""
    lo = max(0, -(-(r - (K - 1)) // 2))   # ceil((r-4)/2)
    hi = min(NO - 1, r // 2)
    return lo, hi


def build_schedule():
    """Per-phase block lists. Block cols are offsets into the packed w stream.

    Returns (phases, totcols, wmax) where phases[mp] is a list of dicts:
      kind 'main': K=128 block, taps j=(0,1) for ow=mp [role 0] and/or
                   j=(2,3) for ow=mp-1 [role 1]; ncols = noh*nroles*64.
      kind 'j4':   vertically-paired K=64 blocks sharing one column range:
                   rows 0:64   = tap j=4 for ow=2t   (even slot),
                   rows 64:128 = tap j=4 for ow=2t+1 (odd slot),
                   t = (mp-2)//2, emitted on even phases mp=2..30;
                   ncols = noh*64.
    """
    phases = []
    col = 0
    wmax = 0
    for mp in range(NMP):
        blocks = []
        for r in range(NR):
            if mp <= OW:  # main blocks exist for mp=0..30
                roles = []
                if mp <= OW - 1:
                    roles.append(0)          # a1: loc (oh, mp), j in {0,1}
                if mp >= 1:
                    roles.append(1)          # a2: loc (oh, mp-1), j in {2,3}
                lo, hi = _oh_span(r)
                if roles and lo <= hi:
                    ncols = (hi - lo + 1) * len(roles) * U
                    blocks.append(dict(kind="main", r=r, mp=mp, col0=col,
                                       ncols=ncols, oh0=lo, noh=hi - lo + 1,
                                       roles=tuple(roles)))
                    col += ncols
        if mp >= 2 and mp % 2 == 0:
            t = (mp - 2) // 2                # pair covers ow = 2t, 2t+1
            for r in range(NR):
                lo, hi = _oh_span(r)
                if lo <= hi:
                    ncols = (hi - lo + 1) * U
                    blocks.append(dict(kind="j4", r=r, mp=mp, col0=col,
                                       ncols=ncols, oh0=lo, noh=hi - lo + 1,
                                       t=t))
                    col += ncols
        pc = sum(bl["ncols"] for bl in blocks)
        wmax = max(wmax, pc)
        phases.append(blocks)
    return phases, col, wmax


W_SCALE = 32.0  # w stream is stored as float8_e3m4 * 32; host divides out


def pack_inputs(x, w, b):
    """Build the per-core input arrays. Returns list of dicts for in_maps."""
    import ml_dtypes

    x = np.ascontiguousarray(np.asarray(x, dtype=np.float32))
    w = np.asarray(w, dtype=np.float32)
    b = np.asarray(b, dtype=np.float32)

    phases, totcols, _ = build_schedule()

    # x: pad rows to HP, transpose to [h, w, c, b] fp16
    xT = np.zeros((HP, W, C, B), dtype=np.float16)
    xT[:H] = x.transpose(1, 2, 3, 0).astype(np.float16)

    # w: [OH,OW,U,K,K,C] -> wt [OHP, OW, K(i), K(j), C, U] fp32, padded oh rows
    wt = np.zeros((OHP, OW, K, K, C, U), dtype=np.float32)
    wt[:OH] = w.transpose(0, 1, 3, 4, 5, 2)

    # bias carries the W_SCALE so psum accumulates W_SCALE*(conv+bias);
    # the host gather divides it back out.
    bias_pad = np.zeros((OHP, OW, U), dtype=np.float32)
    bias_pad[:OH] = b * W_SCALE

    in_maps = []
    for core in range(NCORES):
        oh0 = core * NO
        r0 = 2 * oh0
        # x tile: [128, NMP*NR*B]; free = (mp, r, b).
        # Partition halves are PARITY-SWAPPED: tile mp holds its even column
        # (2mp) in partitions 0:64 when mp is even, in partitions 64:128 when
        # mp is odd. This gives j4 matmuls a base-64 lhsT for odd tiles so
        # vertically-paired j4 w-blocks need no zero padding and no extra
        # x array.
        xc = xT[r0:r0 + NR]                                  # [NR, W, C, B]
        xc = xc.reshape(NR, NMP, 2, C, B)                    # [r, mp, dj, c, b]
        xc = xc.transpose(1, 2, 3, 0, 4)                     # [mp, dj, c, r, b]
        xc = xc.copy()
        xc[1::2] = xc[1::2, ::-1]                            # swap halves, odd mp
        xtile = np.ascontiguousarray(
            xc.transpose(1, 2, 0, 3, 4).reshape(128, NMP * NR * B))

        # w stream (built fp32, quantized to e3m4 at the end)
        ws = np.zeros((128, totcols), dtype=np.float32)
        for mp, blocks in enumerate(phases):
            flip = (mp % 2 == 1)
            for bl in blocks:
                r = bl["r"]
                lo, noh = bl["oh0"], bl["noh"]
                ohs = np.arange(lo, lo + noh)
                i_s = r - 2 * ohs
                ohs_g = oh0 + ohs
                c0 = bl["col0"]
                if bl["kind"] == "main":
                    for k, role in enumerate(bl["roles"]):
                        ow = mp if role == 0 else mp - 1
                        j0 = 0 if role == 0 else 2
                        # [noh, 2(dj), C, U]
                        src = wt[ohs_g, ow, i_s, j0:j0 + 2]
                        if flip:
                            src = src[:, ::-1]
                        # -> [128=(dj,c), noh, U] -> per-(oh,role) col chunks
                        blk = src.transpose(1, 2, 0, 3).reshape(128, noh, U)
                        nroles = len(bl["roles"])
                        for t in range(noh):
                            cc = c0 + (t * nroles + k) * U
                            ws[:, cc:cc + U] = blk[:, t, :]
                else:
                    t = bl["t"]
                    for dh, ow4 in ((0, 2 * t), (1, 2 * t + 1)):
                        src = wt[ohs_g, ow4, i_s, 4]         # [noh, C, U]
                        blk = src.transpose(1, 0, 2).reshape(C, noh * U)
                        ws[dh * C:(dh + 1) * C, c0:c0 + bl["ncols"]] = blk

        # bias: [1, OW*NO*U] fp32, (ow, oh_l, u) order; broadcast on device
        bias_1 = np.ascontiguousarray(
            bias_pad[oh0:oh0 + NO].transpose(1, 0, 2).reshape(1, OW * NO * U))

        ws8 = (ws * W_SCALE).astype(ml_dtypes.float8_e3m4)
        in_maps.append({"xt": xtile, "wstream": ws8, "bias_1": bias_1})
    return in_maps


def emulate_core(inp):
    """Numpy emulation of the device program for one core (validation)."""
    phases, totcols, _ = build_schedule()
    xt = inp["xt"].astype(np.float32)
    ws = inp["wstream"].astype(np.float32)
    bias = np.broadcast_to(inp["bias_1"], (64, OW * NO * U))
    pts = {}
    stag = np.zeros((64, OW, NO, U), dtype=np.float32)
    for mp, blocks in enumerate(phases):
        if mp <= OW:
            pts[mp] = np.zeros((64, NO, 2, U), dtype=np.float32)
        for bl in blocks:
            r = bl["r"]
            lo, noh = bl["oh0"], bl["noh"]
            rhs = ws[:, bl["col0"]:bl["col0"] + bl["ncols"]]
            if bl["kind"] == "main":
                xoff = (mp * NR + r) * B
                lhsT = xt[:, xoff:xoff + B]
                res = lhsT.T @ rhs                       # [64, noh*nroles*64]
                res = res.reshape(64, noh, len(bl["roles"]), U)
                for k, role in enumerate(bl["roles"]):
                    pts[mp][:, lo:lo + noh, role, :] += res[:, :, k, :]
            else:
                t = bl["t"]
                # even slot: tile mp (normal), partitions 0:64
                xoff = (mp * NR + r) * B
                lhsT = xt[0:C, xoff:xoff + B]
                res = lhsT.T @ rhs[0:C]
                pts[2 * t][:, lo:lo + noh, 0, :] += res.reshape(64, noh, U)
                # odd slot: tile mp+1 (swapped), partitions 64:128
                xoff = ((mp + 1) * NR + r) * B
                lhsT = xt[C:2 * C, xoff:xoff + B]
                res = lhsT.T @ rhs[C:2 * C]
                pts[2 * t + 1][:, lo:lo + noh, 0, :] += res.reshape(64, noh, U)
        ow = mp - 2
        if 0 <= ow <= OW - 1:
            a1 = pts[ow][:, :, 0, :]
            a2 = pts[ow + 1][:, :, 1, :]
            bv = bias[:, ow * NO * U:(ow + 1) * NO * U].reshape(64, NO, U)
            stag[:, ow] = a1 + bv + a2
    return stag / W_SCALE  # [64, ow, oh_l, u]


# ---------------- device kernel ----------------

def build_nc(loop_n=1):
    """Build the device program. loop_n > 1 wraps the whole phase sweep in a
    hardware For_i loop (identical work each iteration) — used only to
    measure per-iteration HW time above the RPC noise floor."""
    import concourse.bass as bass  # noqa: F401
    import concourse.mybir as mybir
    import concourse.tile as tile
    from concourse import bacc

    phases, totcols, wmax = build_schedule()
    dt = mybir.dt

    ablate = os.environ.get("KABLATE", "")  # dev-only: "nomm","nodve","nodma"
    nc = bacc.Bacc("TRN2", target_bir_lowering=False, debug=False,
                   num_devices=NCORES)
    xt_d = nc.dram_tensor("xt", [128, NMP * NR * B], dt.float16,
                          kind="ExternalInput").ap()
    ws_d = nc.dram_tensor("wstream", [128, totcols], dt.float8e3,
                          kind="ExternalInput").ap()
    bias_d = nc.dram_tensor("bias_1", [1, OW * NO * U], dt.float32,
                            kind="ExternalInput").ap()
    out_d = nc.dram_tensor("out", [B, OW, NO, U], dt.bfloat16,
                           kind="ExternalOutput").ap()

    with tile.TileContext(nc) as tc:
        with tc.tile_pool(name="xpool", bufs=1) as xpool, \
             tc.tile_pool(name="bpool", bufs=1) as bpool, \
             tc.tile_pool(name="stpool", bufs=1) as stpool, \
             tc.tile_pool(name="wpool", bufs=int(os.environ.get("WBUFS","8"))) as wpool, \
             tc.tile_pool(name="tmppool", bufs=4) as tmppool, \
             tc.tile_pool(name="pspool", bufs=int(os.environ.get("PSBUFS","5")), space="PSUM") as pspool:

            # Two HWDGE rings: w phase DMAs alternate between them; the x
            # preload is chunked on the ACT ring so early w phases can start
            # while later x chunks stream in.
            dma_w = nc.sync
            dma_x = nc.scalar

            xsb = xpool.tile([128, NMP * NR * B], dt.float16, tag="xt")
            XCH = int(os.environ.get("XCH", "4"))  # x chunks
            xch = NMP // XCH * NR * B
            for g in range(XCH):
                dma_x.dma_start(xsb[:, g * xch:(g + 1) * xch],
                                xt_d[:, g * xch:(g + 1) * xch])
            b1 = bpool.tile([1, OW * NO * U], dt.float32, tag="b1")
            dma_x.dma_start(b1[:, :], bias_d[:, :])
            bsb = bpool.tile([64, OW * NO * U], dt.float32, tag="brep")
            nc.gpsimd.partition_broadcast(bsb[:, :], b1[:, :], channels=64)
            stag = stpool.tile([64, OW * NO * U], dt.bfloat16)

            import contextlib
            loop_cm = (tc.For_i(0, loop_n, 1) if loop_n > 1
                       else contextlib.nullcontext())
            with loop_cm:
                _emit_sweep(nc, tc, phases, wmax, dt, ablate, dma_w, dma_x,
                            xsb, bsb, stag, ws_d, out_d,
                            wpool, tmppool, pspool)

    nc.compile()
    return nc


def _emit_sweep(nc, tc, phases, wmax, dt, ablate, dma_w, dma_x,
                xsb, bsb, stag, ws_d, out_d, wpool, tmppool, pspool):
    pts = {}
    for mp, blocks in enumerate(phases):
        wcols = sum(bl["ncols"] for bl in blocks)
        if wcols:
            pc0 = blocks[0]["col0"]
            wsb = wpool.tile([128, wmax], dt.float8e3, tag="wstream")
            if ablate != "nodma":
                ring = dma_w if mp % 2 == 0 else dma_x
                ring.dma_start(wsb[:, :wcols],
                               ws_d[:, pc0:pc0 + wcols])

        use_memset = os.environ.get("KMEMSET", "0") == "1"
        if mp <= OW:
            pt = pspool.tile([64, NO * 2 * U], dt.float32)
            pts[mp] = pt
            if use_memset and ablate != "nodve":
                nc.vector.memset(pt[:, :], 0.0)

        for bl in blocks:
            r = bl["r"]
            lo, noh = bl["oh0"], bl["noh"]
            loc0 = bl["col0"] - pc0
            if bl["kind"] == "main":
                xoff = (mp * NR + r) * B
                ptv = pts[mp][:, :].rearrange(
                    "p (o q) -> p o q", o=NO, q=2 * U)
                lhsT = xsb[:, xoff:xoff + B]
                nroles = len(bl["roles"])
                # The first write to each (oh, role) slot uses start=True
                # (clears has_written -> overwrite), replacing the DVE
                # memset. Within a phase the r-loop covers oh
                # monotonically: slot oh is first written at r = 2*oh.
                newoh = (r // 2 if (r % 2 == 0 and r // 2 <= NO - 1
                                   and not use_memset) else None)
                parts = []          # (oh_lo, cnt, start)
                if newoh is not None:
                    assert newoh == lo + noh - 1
                    if noh > 1:
                        parts.append((lo, noh - 1, False))
                    parts.append((newoh, 1, True))
                else:
                    parts.append((lo, noh, False))
                for p_lo, p_n, p_start in parts:
                    cofs = loc0 + (p_lo - lo) * nroles * U
                    rhs = wsb[:, cofs:cofs + p_n * nroles * U]
                    if nroles == 2:
                        outap = ptv[:, p_lo:p_lo + p_n, :]
                    elif bl["roles"][0] == 0:
                        outap = ptv[:, p_lo:p_lo + p_n, 0:U]
                    else:
                        outap = ptv[:, p_lo:p_lo + p_n, U:2 * U]
                    if ablate != "nomm":
                        nc.tensor.matmul(outap, lhsT, rhs, start=p_start,
                                         stop=False, skip_group_check=True)
            else:
                t = bl["t"]
                # even slot: tile mp (normal parity), base 0
                # odd slot:  tile mp+1 (swapped), base 64
                for dh, ow4, tmp_mp in ((0, 2 * t, mp),
                                        (1, 2 * t + 1, mp + 1)):
                    xoff = (tmp_mp * NR + r) * B
                    ptv = pts[ow4][:, :].rearrange(
                        "p (o q) -> p o q", o=NO, q=2 * U)
                    lhsT = xsb[dh * C:(dh + 1) * C, xoff:xoff + B]
                    rhs = wsb[dh * C:(dh + 1) * C,
                              loc0:loc0 + bl["ncols"]]
                    outap = ptv[:, lo:lo + noh, 0:U]
                    if ablate != "nomm":
                        nc.tensor.matmul(outap, lhsT, rhs, start=False,
                                         stop=False,
                                         skip_group_check=True)

        ow = mp - 2
        if 0 <= ow <= OW - 1:
            a1 = pts[ow][:, :].rearrange(
                "p (o q) -> p o q", o=NO, q=2 * U)[:, :, 0:U]
            a2 = pts[ow + 1][:, :].rearrange(
                "p (o q) -> p o q", o=NO, q=2 * U)[:, :, U:2 * U]
            bv = bsb[:, ow * NO * U:(ow + 1) * NO * U].rearrange(
                "p (o u) -> p o u", u=U)
            tmp = tmppool.tile([64, NO * U], dt.float32)
            tmpv = tmp[:, :].rearrange("p (o u) -> p o u", u=U)
            stv = stag[:, ow * NO * U:(ow + 1) * NO * U].rearrange(
                "p (o u) -> p o u", u=U)
            if ablate != "nodve":
                nc.vector.tensor_add(tmpv, a1, bv)
                nc.vector.tensor_add(stv, tmpv, a2)
            del pts[ow]
            # stream the output out as rows complete: 8-ow chunks early,
            # then 2-ow chunks so the tail DMAs overlap the final drains
            if ow < 24 and ow % 8 == 7:
                g = ow // 8
                sl = slice(g * 8 * NO * U, (g + 1) * 8 * NO * U)
                dma_w.dma_start(
                    out_d.rearrange("b w o u -> b (w o u)")[:, sl],
                    stag[:, sl])
            elif ow >= 24 and ow % 2 == 1:
                sl = slice((ow - 1) * NO * U, (ow + 1) * NO * U)
                dma_w.dma_start(
                    out_d.rearrange("b w o u -> b (w o u)")[:, sl],
                    stag[:, sl])


def _exec(nc, in_maps, repeats=1, chain=1):
    """Execute the prebuilt Bass module on the 8 cores via PJRT/axon.

    Mirrors bass2jax.run_bass_via_pjrt's multi-core branch, but keeps the
    jitted executable + device-staged inputs so the kernel can be re-run for
    timing. `chain` repeats the kernel execution inside one program (for
    amortized on-device timing). Returns (per_core_results, wall_times_s).
    """
    import time

    import jax
    import numpy as _np
    from jax.sharding import Mesh, NamedSharding, PartitionSpec

    try:
        from jax.experimental.shard_map import shard_map
    except ImportError:
        from jax.shard_map import shard_map

    import concourse.mybir as mybir
    from concourse import bass2jax

    bass2jax.install_neuronx_cc_hook()

    partition_name = (nc.partition_id_tensor.name
                      if nc.partition_id_tensor else None)
    in_names, out_names, out_avals, zero_outs = [], [], [], []
    for alloc in nc.m.functions[0].allocations:
        if not isinstance(alloc, mybir.MemoryLocationSet):
            continue
        name = alloc.memorylocations[0].name
        if alloc.kind == "ExternalInput":
            if name != partition_name:
                in_names.append(name)
        elif alloc.kind == "ExternalOutput":
            out_names.append(name)
            shape = tuple(alloc.tensor_shape)
            dtype = mybir.dt.np(alloc.dtype)
            out_avals.append(jax.core.ShapedArray(shape, dtype))
            zero_outs.append(_np.zeros(shape, dtype))
    n_params = len(in_names)
    all_names = in_names + out_names
    if partition_name is not None:
        all_names = all_names + [partition_name]

    def _bind(operands):
        return bass2jax._bass_exec_p.bind(
            *operands,
            out_avals=tuple(out_avals),
            in_names=tuple(all_names),
            out_names=tuple(out_names),
            lowering_input_output_aliases=(),
            sim_require_finite=True,
            sim_require_nnan=True,
            nc=nc,
        )

    def _body(*args):
        operands = list(args)
        if partition_name is not None:
            operands.append(bass2jax.partition_id_tensor())
        return tuple(_bind(operands))

    n_cores = len(in_maps)
    devices = jax.devices()[:n_cores]
    mesh = Mesh(_np.asarray(devices), ("core",))
    spec = PartitionSpec("core")
    sharded = jax.jit(
        shard_map(_body, mesh=mesh, in_specs=(spec,) * (n_params + len(out_names)),
                  out_specs=(spec,) * len(out_names), check_rep=False),
        keep_unused=True,
    )
    sharding = NamedSharding(mesh, spec)
    staged = [
        jax.device_put(
            _np.concatenate([_np.asarray(m[name]) for m in in_maps], axis=0),
            sharding)
        for name in in_names
    ] + [
        jax.device_put(
            _np.zeros((n_cores * z.shape[0], *z.shape[1:]), z.dtype), sharding)
        for z in zero_outs
    ]

    times = []
    out_arrs = None
    for _ in range(max(1, repeats)):
        t0 = time.perf_counter()
        out_arrs = jax.block_until_ready(sharded(*staged))
        times.append(time.perf_counter() - t0)

    results = [
        {
            name: _np.asarray(out_arrs[i]).reshape(n_cores, *out_avals[i].shape)[c]
            for i, name in enumerate(out_names)
        }
        for c in range(n_cores)
    ]
    return results, times


def _run(inputs, repeats=1):
    """Run on hardware. Returns (full_output, wall_times_s)."""
    in_maps = pack_inputs(inputs["x"], inputs["w"], inputs["b"])
    nc = build_nc()
    results, times = _exec(nc, in_maps, repeats=repeats)
    return _gather(results), times


def _gather(results):
    out = np.empty((B, OHP, OW, U), dtype=np.float32)
    for c in range(NCORES):
        # per-core out [B, OW, NO, U] bf16 (scaled by W_SCALE)
        out[:, c * NO:(c + 1) * NO] = (
            results[c]["out"].astype(np.float32).transpose(0, 2, 1, 3))
    return out[:, :OH] * (1.0 / W_SCALE)


def kernel(x, w, b):
    from concourse.bass_utils import run_bass_kernel_spmd

    in_maps = pack_inputs(x, w, b)
    nc = build_nc()
    res = run_bass_kernel_spmd(nc, in_maps, list(range(NCORES)))
    return _gather(res.results)



# revision 17
# speedup vs baseline: 1471.1406x; 1471.1406x over previous
"""FreeConv2D (locally-connected conv2d + bias) Trainium2 Bass kernel.

out[b,oh,ow,u] = sum_{i,j,c} w[oh,ow,u,i,j,c] * x[b, oh*2+i, ow*2+j, c] + bias[oh,ow,u]

Shapes: x [64,64,64,64], w [30,30,64,5,5,64], b [30,30,64] -> out [64,30,30,64].

Strategy (8 NeuronCores):
  - Shard output rows OH over cores: 4 rows/core (padded 30->32; last 2 dummy).
  - The kernel is DMA-bound (~330 GB/s/core aggregate): w dominates traffic,
    so the w stream is stored as float8_e3m4 * 32 (half the bytes of fp16;
    measured rel err ~1.1e-2 vs the 2e-2 gate) and the output as bf16. The
    matmul mixes lhsT fp16 (x) with rhs fp8e3 (w) — allowed on TRN2.
  - PSUM tiles are not memset: the first matmul into each (oh, role) slot
    uses start=True (even-r blocks are split so the fresh-oh part is its own
    matmul), which keeps the DVE free for drains.
  - Host pre-packs (numpy, not counted in HW time):
      * x    -> per-core fp16 tile [128, 11*32*64]: partition p = dj*64+c for
               column pair (2*mp, 2*mp+1), free = (r, mp, b).
      * w    -> per-core fp8e3 stream [128, TOT] (values * 32): matmul rhs
               blocks in execution order (column-pair taps j in {0,1} / {2,3}
               as K=128 blocks; j=4 taps as K=64 vertically-paired blocks).
      * bias -> per-core fp32 [64, 30*4*64] * 32 replicated over batch
               partitions; host gather divides the 32 back out.
  - Device: 32-phase sweep over column pairs mp. Phase mp:
      * DMA the phase's w blocks (~1 MB).
      * psum phase tile pt[mp] [64, 512] = accum slots (oh_l, role) where
        role 0 = j01-half of loc (oh, mp), role 1 = j23-half of loc (oh, mp-1).
      * matmuls: lhsT = resident x tile [128, 64(b)] (stationary),
        rhs = w blocks [128, N<=512] (moving), accumulate with start=False
        (tiles pre-zeroed by DVE memset; psum has_written semantics make this
        correct whether the first PE write accumulates or overwrites).
      * j=4 taps (K=64) of loc (oh, mp-2) also land in pt[mp-2] role-0 slots.
      * drain loc (.., ow=mp-2): out = pt[ow].role0 + bias + pt[ow+1].role1
        via two DVE tensor_adds into an SBUF staging buffer.
  - One final DMA of staging -> DRAM out [64, 30(ow), 4(oh_l), 64] per core;
    host gathers/transposes/trims to [64, 30, 30, 64].
"""

import os
import sys

import numpy as np

_TRN_REPO = "/opt/trn_rl_repo"
if _TRN_REPO not in sys.path:
    sys.path.insert(0, _TRN_REPO)

# The kernel needs the axon/neuron jax backend; a JAX_PLATFORMS=cpu pin (used
# for reference computation) would hide the NeuronCores. Only effective if jax
# has not been initialized yet in this process.
if "jax" not in sys.modules and "axon" not in os.environ.get("JAX_PLATFORMS", "axon"):
    os.environ.pop("JAX_PLATFORMS", None)

# ---------------- problem constants (hardcoded) ----------------
B, H, W, C = 64, 64, 64, 64
U, K, S = 64, 5, 2
OH = OW = 30
NCORES = 8
NO = 4                      # oh rows per core (padded: 8*4 = 32 >= 30)
OHP = NCORES * NO           # 32
NR = 2 * (NO - 1) + K       # 11 input rows per core
NMP = 32                    # column-pair tiles mp=0..31; also phase count
NT4 = OW // 2               # 15 j4 ow-pairs
HP = 2 * (OHP - 1) + K      # 67 padded input rows overall


def _oh_span(r):
    """Valid local oh range for local input row r: i = r - 2*oh in [0, K-1]."""
    lo = max(0, -(-(r - (K - 1)) // 2))   # ceil((r-4)/2)
    hi = min(NO - 1, r // 2)
    return lo, hi


def build_schedule():
    """Per-phase block lists. Block cols are offsets into the packed w stream.

    Accumulation is single-slot: each output column ow owns one PSUM tile
    PS[ow] [64, NO*U]; every matmul targets the owning tile directly.

    Returns (phases, totcols, wmax) where phases[mp] is a list of dicts:
      kind 'main': K=128 block; role 0 = taps j=(0,1) for ow=mp,
                   role 1 = taps j=(2,3) for ow=mp-1 (separate blocks, each
                   targeting PS[ow]); ncols = noh*64.
      kind 'j4':   vertically-paired K=64 blocks sharing one column range:
                   rows 0:64   = tap j=4 for ow=2t   (even slot),
                   rows 64:128 = tap j=4 for ow=2t+1 (odd slot),
                   t = (mp-2)//2, emitted on even phases mp=2..30;
                   ncols = noh*64.
    """
    phases = []
    col = 0
    wmax = 0
    for mp in range(NMP):
        blocks = []
        for r in range(NR):
            if mp <= OW:  # main blocks exist for mp=0..30
                lo, hi = _oh_span(r)
                if lo > hi:
                    continue
                noh = hi - lo + 1
                for role in (0, 1):
                    ow = mp - role
                    if not (0 <= ow <= OW - 1):
                        continue
                    blocks.append(dict(kind="main", r=r, mp=mp, col0=col,
                                       ncols=noh * U, oh0=lo, noh=noh,
                                       role=role, ow=ow))
                    col += noh * U
        if mp >= 2 and mp % 2 == 0:
            t = (mp - 2) // 2                # pair covers ow = 2t, 2t+1
            for r in range(NR):
                lo, hi = _oh_span(r)
                if lo <= hi:
                    ncols = (hi - lo + 1) * U
                    blocks.append(dict(kind="j4", r=r, mp=mp, col0=col,
                                       ncols=ncols, oh0=lo, noh=hi - lo + 1,
                                       t=t))
                    col += ncols
        pc = sum(bl["ncols"] for bl in blocks)
        wmax = max(wmax, pc)
        phases.append(blocks)
    return phases, col, wmax


W_SCALE = 32.0  # w stream is stored as float8_e3m4 * 32; host divides out


def pack_inputs(x, w, b):
    """Build the per-core input arrays. Returns list of dicts for in_maps."""
    import ml_dtypes

    x = np.ascontiguousarray(np.asarray(x, dtype=np.float32))
    w = np.asarray(w, dtype=np.float32)
    b = np.asarray(b, dtype=np.float32)

    phases, totcols, _ = build_schedule()

    # x: pad rows to HP, transpose to [h, w, c, b] fp16
    xT = np.zeros((HP, W, C, B), dtype=np.float16)
    xT[:H] = x.transpose(1, 2, 3, 0).astype(np.float16)

    # w: [OH,OW,U,K,K,C] -> wt [OHP, OW, K(i), K(j), C, U] fp32, padded oh rows
    wt = np.zeros((OHP, OW, K, K, C, U), dtype=np.float32)
    wt[:OH] = w.transpose(0, 1, 3, 4, 5, 2)

    # bias carries the W_SCALE so psum accumulates W_SCALE*(conv+bias);
    # the host gather divides it back out.
    bias_pad = np.zeros((OHP, OW, U), dtype=np.float32)
    bias_pad[:OH] = b * W_SCALE

    in_maps = []
    for core in range(NCORES):
        oh0 = core * NO
        r0 = 2 * oh0
        # x tile: [128, NMP*NR*B]; free = (mp, r, b).
        # Partition halves are PARITY-SWAPPED: tile mp holds its even column
        # (2mp) in partitions 0:64 when mp is even, in partitions 64:128 when
        # mp is odd. This gives j4 matmuls a base-64 lhsT for odd tiles so
        # vertically-paired j4 w-blocks need no zero padding and no extra
        # x array.
        xc = xT[r0:r0 + NR]                                  # [NR, W, C, B]
        xc = xc.reshape(NR, NMP, 2, C, B)                    # [r, mp, dj, c, b]
        xc = xc.transpose(1, 2, 3, 0, 4)                     # [mp, dj, c, r, b]
        xc = xc.copy()
        xc[1::2] = xc[1::2, ::-1]                            # swap halves, odd mp
        xtile = np.ascontiguousarray(
            xc.transpose(1, 2, 0, 3, 4).reshape(128, NMP * NR * B))

        # w stream (built fp32, quantized to e3m4 at the end)
        ws = np.zeros((128, totcols), dtype=np.float32)
        for mp, blocks in enumerate(phases):
            flip = (mp % 2 == 1)
            for bl in blocks:
                r = bl["r"]
                lo, noh = bl["oh0"], bl["noh"]
                ohs = np.arange(lo, lo + noh)
                i_s = r - 2 * ohs
                ohs_g = oh0 + ohs
                c0 = bl["col0"]
                if bl["kind"] == "main":
                    ow, role = bl["ow"], bl["role"]
                    j0 = 0 if role == 0 else 2
                    # [noh, 2(dj), C, U]
                    src = wt[ohs_g, ow, i_s, j0:j0 + 2]
                    if flip:
                        src = src[:, ::-1]
                    # -> [128=(dj,c), noh*U] oh-major col chunks
                    blk = src.transpose(1, 2, 0, 3).reshape(128, noh * U)
                    ws[:, c0:c0 + noh * U] = blk
                else:
                    t = bl["t"]
                    for dh, ow4 in ((0, 2 * t), (1, 2 * t + 1)):
                        src = wt[ohs_g, ow4, i_s, 4]         # [noh, C, U]
                        blk = src.transpose(1, 0, 2).reshape(C, noh * U)
                        ws[dh * C:(dh + 1) * C, c0:c0 + bl["ncols"]] = blk

        # bias: [1, OW*NO*U] fp32, (ow, oh_l, u) order; broadcast on device
        bias_1 = np.ascontiguousarray(
            bias_pad[oh0:oh0 + NO].transpose(1, 0, 2).reshape(1, OW * NO * U))

        ws8 = (ws * W_SCALE).astype(ml_dtypes.float8_e3m4)
        in_maps.append({"xt": xtile, "wstream": ws8, "bias_1": bias_1})
    return in_maps


def emulate_core(inp):
    """Numpy emulation of the device program for one core (validation)."""
    phases, totcols, _ = build_schedule()
    xt = inp["xt"].astype(np.float32)
    ws = inp["wstream"].astype(np.float32)
    bias = np.broadcast_to(inp["bias_1"], (64, OW * NO * U))
    pts = {}
    stag = np.zeros((64, OW, NO, U), dtype=np.float32)
    for mp, blocks in enumerate(phases):
        if mp <= OW - 1:
            pts[mp] = np.zeros((64, NO, U), dtype=np.float32)
        for bl in blocks:
            r = bl["r"]
            lo, noh = bl["oh0"], bl["noh"]
            rhs = ws[:, bl["col0"]:bl["col0"] + bl["ncols"]]
            if bl["kind"] == "main":
                xoff = (mp * NR + r) * B
                lhsT = xt[:, xoff:xoff + B]
                res = lhsT.T @ rhs                       # [64, noh*64]
                pts[bl["ow"]][:, lo:lo + noh, :] += res.reshape(64, noh, U)
            else:
                t = bl["t"]
                # even slot: tile mp (normal), partitions 0:64
                xoff = (mp * NR + r) * B
                lhsT = xt[0:C, xoff:xoff + B]
                res = lhsT.T @ rhs[0:C]
                pts[2 * t][:, lo:lo + noh, :] += res.reshape(64, noh, U)
                # odd slot: tile mp+1 (swapped), partitions 64:128
                xoff = ((mp + 1) * NR + r) * B
                lhsT = xt[C:2 * C, xoff:xoff + B]
                res = lhsT.T @ rhs[C:2 * C]
                pts[2 * t + 1][:, lo:lo + noh, :] += res.reshape(64, noh, U)
        ow = mp - 2
        if 0 <= ow <= OW - 1:
            bv = bias[:, ow * NO * U:(ow + 1) * NO * U].reshape(64, NO, U)
            stag[:, ow] = pts.pop(ow) + bv
    return stag / W_SCALE  # [64, ow, oh_l, u]


# ---------------- device kernel ----------------

def build_nc(loop_n=1):
    """Build the device program. loop_n > 1 wraps the whole phase sweep in a
    hardware For_i loop (identical work each iteration) — used only to
    measure per-iteration HW time above the RPC noise floor."""
    import concourse.bass as bass  # noqa: F401
    import concourse.mybir as mybir
    import concourse.tile as tile
    from concourse import bacc

    phases, totcols, wmax = build_schedule()
    dt = mybir.dt

    ablate = os.environ.get("KABLATE", "")  # dev-only: "nomm","nodve","nodma"
    nc = bacc.Bacc("TRN2", target_bir_lowering=False, debug=False,
                   num_devices=NCORES)
    xt_d = nc.dram_tensor("xt", [128, NMP * NR * B], dt.float16,
                          kind="ExternalInput").ap()
    ws_d = nc.dram_tensor("wstream", [128, totcols], dt.float8e3,
                          kind="ExternalInput").ap()
    bias_d = nc.dram_tensor("bias_1", [1, OW * NO * U], dt.float32,
                            kind="ExternalInput").ap()
    out_d = nc.dram_tensor("out", [B, OW, NO, U], dt.bfloat16,
                           kind="ExternalOutput").ap()

    with tile.TileContext(nc) as tc:
        with tc.tile_pool(name="xpool", bufs=1) as xpool, \
             tc.tile_pool(name="bpool", bufs=1) as bpool, \
             tc.tile_pool(name="stpool", bufs=1) as stpool, \
             tc.tile_pool(name="wpool", bufs=int(os.environ.get("WBUFS","8"))) as wpool, \
             tc.tile_pool(name="tmppool", bufs=4) as tmppool, \
             tc.tile_pool(name="pspool", bufs=int(os.environ.get("PSBUFS","5")), space="PSUM") as pspool:

            # Two HWDGE rings: w phase DMAs alternate between them; the x
            # preload is chunked on the ACT ring so early w phases can start
            # while later x chunks stream in.
            dma_w = nc.sync
            dma_x = nc.scalar

            xsb = xpool.tile([128, NMP * NR * B], dt.float16, tag="xt")
            XCH = int(os.environ.get("XCH", "4"))  # x chunks
            xch = NMP // XCH * NR * B
            for g in range(XCH):
                dma_x.dma_start(xsb[:, g * xch:(g + 1) * xch],
                                xt_d[:, g * xch:(g + 1) * xch])
            b1 = bpool.tile([1, OW * NO * U], dt.float32, tag="b1")
            dma_x.dma_start(b1[:, :], bias_d[:, :])
            bsb = bpool.tile([64, OW * NO * U], dt.float32, tag="brep")
            nc.gpsimd.partition_broadcast(bsb[:, :], b1[:, :], channels=64)
            stag = stpool.tile([64, OW * NO * U], dt.bfloat16)

            import contextlib
            loop_cm = (tc.For_i(0, loop_n, 1) if loop_n > 1
                       else contextlib.nullcontext())
            with loop_cm:
                _emit_sweep(nc, tc, phases, wmax, dt, ablate, dma_w, dma_x,
                            xsb, bsb, stag, ws_d, out_d,
                            wpool, tmppool, pspool)

    nc.compile()
    return nc


def _emit_sweep(nc, tc, phases, wmax, dt, ablate, dma_w, dma_x,
                xsb, bsb, stag, ws_d, out_d, wpool, tmppool, pspool):
    pts = {}
    for mp, blocks in enumerate(phases):
        wcols = sum(bl["ncols"] for bl in blocks)
        if wcols:
            pc0 = blocks[0]["col0"]
            wsb = wpool.tile([128, wmax], dt.float8e3, tag="wstream")
            if ablate != "nodma":
                ring = dma_w if mp % 2 == 0 else dma_x
                ring.dma_start(wsb[:, :wcols],
                               ws_d[:, pc0:pc0 + wcols])

        if mp <= OW - 1:
            pt = pspool.tile([64, NO * U], dt.float32)
            pts[mp] = pt
            if ablate != "nodve":
                nc.vector.memset(pt[:, :], 0.0)

        for bl in blocks:
            r = bl["r"]
            lo, noh = bl["oh0"], bl["noh"]
            loc0 = bl["col0"] - pc0
            if bl["kind"] == "main":
                xoff = (mp * NR + r) * B
                ptv = pts[bl["ow"]][:, :].rearrange(
                    "p (o u) -> p o u", o=NO, u=U)
                lhsT = xsb[:, xoff:xoff + B]
                rhs = wsb[:, loc0:loc0 + bl["ncols"]]
                outap = ptv[:, lo:lo + noh, :]
                if ablate != "nomm":
                    nc.tensor.matmul(outap, lhsT, rhs, start=False,
                                     stop=False, skip_group_check=True)
            else:
                t = bl["t"]
                # even slot: tile mp (normal parity), base 0
                # odd slot:  tile mp+1 (swapped), base 64
                for dh, ow4, tmp_mp in ((0, 2 * t, mp),
                                        (1, 2 * t + 1, mp + 1)):
                    xoff = (tmp_mp * NR + r) * B
                    ptv = pts[ow4][:, :].rearrange(
                        "p (o u) -> p o u", o=NO, u=U)
                    lhsT = xsb[dh * C:(dh + 1) * C, xoff:xoff + B]
                    rhs = wsb[dh * C:(dh + 1) * C,
                              loc0:loc0 + bl["ncols"]]
                    outap = ptv[:, lo:lo + noh, :]
                    if ablate != "nomm":
                        nc.tensor.matmul(outap, lhsT, rhs, start=False,
                                         stop=False,
                                         skip_group_check=True)

        ow = mp - 2
        if 0 <= ow <= OW - 1:
            a1 = pts[ow][:, :].rearrange("p (o u) -> p o u", u=U)
            bv = bsb[:, ow * NO * U:(ow + 1) * NO * U].rearrange(
                "p (o u) -> p o u", u=U)
            stv = stag[:, ow * NO * U:(ow + 1) * NO * U].rearrange(
                "p (o u) -> p o u", u=U)
            if ablate != "nodve":
                nc.vector.tensor_add(stv, a1, bv)
            del pts[ow]
            # stream the output out as rows complete: 8-ow chunks early,
            # then 2-ow chunks so the tail DMAs overlap the final drains
            if ow < 24 and ow % 8 == 7:
                g = ow // 8
                sl = slice(g * 8 * NO * U, (g + 1) * 8 * NO * U)
                dma_w.dma_start(
                    out_d.rearrange("b w o u -> b (w o u)")[:, sl],
                    stag[:, sl])
            elif ow >= 24 and ow % 2 == 1:
                sl = slice((ow - 1) * NO * U, (ow + 1) * NO * U)
                dma_w.dma_start(
                    out_d.rearrange("b w o u -> b (w o u)")[:, sl],
                    stag[:, sl])


def _exec(nc, in_maps, repeats=1, chain=1):
    """Execute the prebuilt Bass module on the 8 cores via PJRT/axon.

    Mirrors bass2jax.run_bass_via_pjrt's multi-core branch, but keeps the
    jitted executable + device-staged inputs so the kernel can be re-run for
    timing. `chain` repeats the kernel execution inside one program (for
    amortized on-device timing). Returns (per_core_results, wall_times_s).
    """
    import time

    import jax
    import numpy as _np
    from jax.sharding import Mesh, NamedSharding, PartitionSpec

    try:
        from jax.experimental.shard_map import shard_map
    except ImportError:
        from jax.shard_map import shard_map

    import concourse.mybir as mybir
    from concourse import bass2jax

    bass2jax.install_neuronx_cc_hook()

    partition_name = (nc.partition_id_tensor.name
                      if nc.partition_id_tensor else None)
    in_names, out_names, out_avals, zero_outs = [], [], [], []
    for alloc in nc.m.functions[0].allocations:
        if not isinstance(alloc, mybir.MemoryLocationSet):
            continue
        name = alloc.memorylocations[0].name
        if alloc.kind == "ExternalInput":
            if name != partition_name:
                in_names.append(name)
        elif alloc.kind == "ExternalOutput":
            out_names.append(name)
            shape = tuple(alloc.tensor_shape)
            dtype = mybir.dt.np(alloc.dtype)
            out_avals.append(jax.core.ShapedArray(shape, dtype))
            zero_outs.append(_np.zeros(shape, dtype))
    n_params = len(in_names)
    all_names = in_names + out_names
    if partition_name is not None:
        all_names = all_names + [partition_name]

    def _bind(operands):
        return bass2jax._bass_exec_p.bind(
            *operands,
            out_avals=tuple(out_avals),
            in_names=tuple(all_names),
            out_names=tuple(out_names),
            lowering_input_output_aliases=(),
            sim_require_finite=True,
            sim_require_nnan=True,
            nc=nc,
        )

    def _body(*args):
        operands = list(args)
        if partition_name is not None:
            operands.append(bass2jax.partition_id_tensor())
        return tuple(_bind(operands))

    n_cores = len(in_maps)
    devices = jax.devices()[:n_cores]
    mesh = Mesh(_np.asarray(devices), ("core",))
    spec = PartitionSpec("core")
    sharded = jax.jit(
        shard_map(_body, mesh=mesh, in_specs=(spec,) * (n_params + len(out_names)),
                  out_specs=(spec,) * len(out_names), check_rep=False),
        keep_unused=True,
    )
    sharding = NamedSharding(mesh, spec)
    staged = [
        jax.device_put(
            _np.concatenate([_np.asarray(m[name]) for m in in_maps], axis=0),
            sharding)
        for name in in_names
    ] + [
        jax.device_put(
            _np.zeros((n_cores * z.shape[0], *z.shape[1:]), z.dtype), sharding)
        for z in zero_outs
    ]

    times = []
    out_arrs = None
    for _ in range(max(1, repeats)):
        t0 = time.perf_counter()
        out_arrs = jax.block_until_ready(sharded(*staged))
        times.append(time.perf_counter() - t0)

    results = [
        {
            name: _np.asarray(out_arrs[i]).reshape(n_cores, *out_avals[i].shape)[c]
            for i, name in enumerate(out_names)
        }
        for c in range(n_cores)
    ]
    return results, times


def _run(inputs, repeats=1):
    """Run on hardware. Returns (full_output, wall_times_s)."""
    in_maps = pack_inputs(inputs["x"], inputs["w"], inputs["b"])
    nc = build_nc()
    results, times = _exec(nc, in_maps, repeats=repeats)
    return _gather(results), times


def _gather(results):
    out = np.empty((B, OHP, OW, U), dtype=np.float32)
    for c in range(NCORES):
        # per-core out [B, OW, NO, U] bf16 (scaled by W_SCALE)
        out[:, c * NO:(c + 1) * NO] = (
            results[c]["out"].astype(np.float32).transpose(0, 2, 1, 3))
    return out[:, :OH] * (1.0 / W_SCALE)


def kernel(x, w, b):
    from concourse.bass_utils import run_bass_kernel_spmd

    in_maps = pack_inputs(x, w, b)
    nc = build_nc()
    res = run_bass_kernel_spmd(nc, in_maps, list(range(NCORES)))
    return _gather(res.results)



# revision 18
# speedup vs baseline: 1986.5670x; 1.3504x over previous
"""FreeConv2D (locally-connected conv2d + bias) Trainium2 Bass kernel.

out[b,oh,ow,u] = sum_{i,j,c} w[oh,ow,u,i,j,c] * x[b, oh*2+i, ow*2+j, c] + bias[oh,ow,u]

Shapes: x [64,64,64,64], w [30,30,64,5,5,64], b [30,30,64] -> out [64,30,30,64].

Strategy (8 NeuronCores):
  - Shard output rows OH over cores: 4 rows/core (padded 30->32; last 2 dummy).
  - The kernel is DMA-bound (~330 GB/s/core aggregate): w dominates traffic,
    so the w stream is stored as float8_e3m4 * 32 (half the bytes of fp16;
    measured rel err ~1.1e-2 vs the 2e-2 gate) and the output as bf16. The
    matmul mixes lhsT fp16 (x) with rhs fp8e3 (w) — allowed on TRN2.
  - PSUM tiles are not memset: the first matmul into each (oh, role) slot
    uses start=True (even-r blocks are split so the fresh-oh part is its own
    matmul), which keeps the DVE free for drains.
  - Host pre-packs (numpy, not counted in HW time):
      * x    -> per-core fp16 tile [128, 11*32*64]: partition p = dj*64+c for
               column pair (2*mp, 2*mp+1), free = (r, mp, b).
      * w    -> per-core fp8e3 stream [128, TOT] (values * 32): matmul rhs
               blocks in execution order (column-pair taps j in {0,1} / {2,3}
               as K=128 blocks; j=4 taps as K=64 vertically-paired blocks).
      * bias -> per-core fp32 [64, 30*4*64] * 32 replicated over batch
               partitions; host gather divides the 32 back out.
  - Device: 32-phase sweep over column pairs mp. Phase mp:
      * DMA the phase's w blocks (~1 MB).
      * psum phase tile pt[mp] [64, 512] = accum slots (oh_l, role) where
        role 0 = j01-half of loc (oh, mp), role 1 = j23-half of loc (oh, mp-1).
      * matmuls: lhsT = resident x tile [128, 64(b)] (stationary),
        rhs = w blocks [128, N<=512] (moving), accumulate with start=False
        (tiles pre-zeroed by DVE memset; psum has_written semantics make this
        correct whether the first PE write accumulates or overwrites).
      * j=4 taps (K=64) of loc (oh, mp-2) also land in pt[mp-2] role-0 slots.
      * drain loc (.., ow=mp-2): out = pt[ow].role0 + bias + pt[ow+1].role1
        via two DVE tensor_adds into an SBUF staging buffer.
  - One final DMA of staging -> DRAM out [64, 30(ow), 4(oh_l), 64] per core;
    host gathers/transposes/trims to [64, 30, 30, 64].
"""

import os
import sys

import numpy as np

_TRN_REPO = "/opt/trn_rl_repo"
if _TRN_REPO not in sys.path:
    sys.path.insert(0, _TRN_REPO)

# The kernel needs the axon/neuron jax backend; a JAX_PLATFORMS=cpu pin (used
# for reference computation) would hide the NeuronCores. Only effective if jax
# has not been initialized yet in this process.
if "jax" not in sys.modules and "axon" not in os.environ.get("JAX_PLATFORMS", "axon"):
    os.environ.pop("JAX_PLATFORMS", None)

# ---------------- problem constants (hardcoded) ----------------
B, H, W, C = 64, 64, 64, 64
U, K, S = 64, 5, 2
OH = OW = 30
NCORES = 8
NO = 4                      # oh rows per core (padded: 8*4 = 32 >= 30)
OHP = NCORES * NO           # 32
NR = 2 * (NO - 1) + K       # 11 input rows per core
NMP = 32                    # column-pair tiles mp=0..31; also phase count
NT4 = OW // 2               # 15 j4 ow-pairs
HP = 2 * (OHP - 1) + K      # 67 padded input rows overall


def _oh_span(r):
    """Valid local oh range for local input row r: i = r - 2*oh in [0, K-1]."""
    lo = max(0, -(-(r - (K - 1)) // 2))   # ceil((r-4)/2)
    hi = min(NO - 1, r // 2)
    return lo, hi


def build_schedule():
    """Per-phase block lists. Block cols are offsets into the packed w stream.

    Accumulation is single-slot: each output column ow owns one PSUM tile
    PS[ow] [64, NO*U]; every matmul targets the owning tile directly.

    Returns (phases, totcols, wmax) where phases[mp] is a list of dicts:
      kind 'main': K=128 block; role 0 = taps j=(0,1) for ow=mp,
                   role 1 = taps j=(2,3) for ow=mp-1 (separate blocks, each
                   targeting PS[ow]); ncols = noh*64.
      kind 'j4m':  K=128 block for tap j=4, row-pairs: partitions
                   (di, c) = input rows (2*rp+di); serves consumers
                   (oh=rp-1, i=2+di) and (oh=rp, i=di) of ow=mp-1, packed
                   oh-ascending in N; lhsT comes from the x4a tile.
      kind 'j4c':  corner tap (i=4, j=4), K=64, vertically paired in the
                   stream: rows 0:64 = w(oh=q, ow=2*gp), rows 64:128 =
                   w(oh=q, ow=2*gp+1); two matmuls (lhsT from x4b halves),
                   emitted on odd phases mp=2*gp+1; ncols = 64.
    """
    phases = []
    col = 0
    wmax = 0
    for mp in range(NMP):
        blocks = []
        for r in range(NR):
            if mp <= OW:  # main blocks exist for mp=0..30
                lo, hi = _oh_span(r)
                if lo > hi:
                    continue
                noh = hi - lo + 1
                for role in (0, 1):
                    ow = mp - role
                    if not (0 <= ow <= OW - 1):
                        continue
                    blocks.append(dict(kind="main", r=r, mp=mp, col0=col,
                                       ncols=noh * U, oh0=lo, noh=noh,
                                       role=role, ow=ow))
                    col += noh * U
        if 1 <= mp <= OW:
            ow = mp - 1
            for rp in range(NO + 1):
                cons = []                    # (oh, i_base), oh ascending
                if 0 <= rp - 1 <= NO - 1:
                    cons.append((rp - 1, 2))
                if rp <= NO - 1:
                    cons.append((rp, 0))
                if cons:
                    ncols = len(cons) * U
                    blocks.append(dict(kind="j4m", rp=rp, mp=mp, col0=col,
                                       ncols=ncols, ow=ow, cons=tuple(cons)))
                    col += ncols
        if mp % 2 == 1 and mp <= OW - 1:
            gp = (mp - 1) // 2               # covers ow = 2*gp, 2*gp+1
            for q in range(NO):
                blocks.append(dict(kind="j4c", q=q, mp=mp, col0=col,
                                   ncols=U, gp=gp))
                col += U
        pc = sum(bl["ncols"] for bl in blocks)
        wmax = max(wmax, pc)
        phases.append(blocks)
    return phases, col, wmax


W_SCALE = 32.0  # w stream is stored as float8_e3m4 * 32; host divides out


def pack_inputs(x, w, b):
    """Build the per-core input arrays. Returns list of dicts for in_maps."""
    import ml_dtypes

    x = np.ascontiguousarray(np.asarray(x, dtype=np.float32))
    w = np.asarray(w, dtype=np.float32)
    b = np.asarray(b, dtype=np.float32)

    phases, totcols, _ = build_schedule()

    # x: pad rows to HP, transpose to [h, w, c, b] fp16
    xT = np.zeros((HP, W, C, B), dtype=np.float16)
    xT[:H] = x.transpose(1, 2, 3, 0).astype(np.float16)

    # w: [OH,OW,U,K,K,C] -> wt [OHP, OW, K(i), K(j), C, U] fp32, padded oh rows
    wt = np.zeros((OHP, OW, K, K, C, U), dtype=np.float32)
    wt[:OH] = w.transpose(0, 1, 3, 4, 5, 2)

    # bias carries the W_SCALE so psum accumulates W_SCALE*(conv+bias);
    # the host gather divides it back out.
    bias_pad = np.zeros((OHP, OW, U), dtype=np.float32)
    bias_pad[:OH] = b * W_SCALE

    in_maps = []
    for core in range(NCORES):
        oh0 = core * NO
        r0 = 2 * oh0
        # x tile: [128, NMP*NR*B]; free = (mp, r, b).
        # Partition halves are PARITY-SWAPPED: tile mp holds its even column
        # (2mp) in partitions 0:64 when mp is even, in partitions 64:128 when
        # mp is odd. This gives j4 matmuls a base-64 lhsT for odd tiles so
        # vertically-paired j4 w-blocks need no zero padding and no extra
        # x array.
        xc = xT[r0:r0 + NR]                                  # [NR, W, C, B]
        xc = xc.reshape(NR, NMP, 2, C, B)                    # [r, mp, dj, c, b]
        xc = xc.transpose(1, 2, 3, 0, 4)                     # [mp, dj, c, r, b]
        xc = xc.copy()
        xc[1::2] = xc[1::2, ::-1]                            # swap halves, odd mp
        xtile = np.ascontiguousarray(
            xc.transpose(1, 2, 0, 3, 4).reshape(128, NMP * NR * B))

        # w stream (built fp32, quantized to e3m4 at the end)
        ws = np.zeros((128, totcols), dtype=np.float32)
        for mp, blocks in enumerate(phases):
            flip = (mp % 2 == 1)
            for bl in blocks:
                r = bl["r"]
                lo, noh = bl["oh0"], bl["noh"]
                ohs = np.arange(lo, lo + noh)
                i_s = r - 2 * ohs
                ohs_g = oh0 + ohs
                c0 = bl["col0"]
                if bl["kind"] == "main":
                    ow, role = bl["ow"], bl["role"]
                    j0 = 0 if role == 0 else 2
                    # [noh, 2(dj), C, U]
                    src = wt[ohs_g, ow, i_s, j0:j0 + 2]
                    if flip:
                        src = src[:, ::-1]
                    # -> [128=(dj,c), noh*U] oh-major col chunks
                    blk = src.transpose(1, 2, 0, 3).reshape(128, noh * U)
                    ws[:, c0:c0 + noh * U] = blk
                else:
                    t = bl["t"]
                    for dh, ow4 in ((0, 2 * t), (1, 2 * t + 1)):
                        src = wt[ohs_g, ow4, i_s, 4]         # [noh, C, U]
                        blk = src.transpose(1, 0, 2).reshape(C, noh * U)
                        ws[dh * C:(dh + 1) * C, c0:c0 + bl["ncols"]] = blk

        # bias: [1, OW*NO*U] fp32, (ow, oh_l, u) order; broadcast on device
        bias_1 = np.ascontiguousarray(
            bias_pad[oh0:oh0 + NO].transpose(1, 0, 2).reshape(1, OW * NO * U))

        ws8 = (ws * W_SCALE).astype(ml_dtypes.float8_e3m4)
        in_maps.append({"xt": xtile, "wstream": ws8, "bias_1": bias_1})
    return in_maps


def emulate_core(inp):
    """Numpy emulation of the device program for one core (validation)."""
    phases, totcols, _ = build_schedule()
    xt = inp["xt"].astype(np.float32)
    ws = inp["wstream"].astype(np.float32)
    bias = np.broadcast_to(inp["bias_1"], (64, OW * NO * U))
    pts = {}
    stag = np.zeros((64, OW, NO, U), dtype=np.float32)
    for mp, blocks in enumerate(phases):
        if mp <= OW - 1:
            pts[mp] = np.zeros((64, NO, U), dtype=np.float32)
        for bl in blocks:
            r = bl["r"]
            lo, noh = bl["oh0"], bl["noh"]
            rhs = ws[:, bl["col0"]:bl["col0"] + bl["ncols"]]
            if bl["kind"] == "main":
                xoff = (mp * NR + r) * B
                lhsT = xt[:, xoff:xoff + B]
                res = lhsT.T @ rhs                       # [64, noh*64]
                pts[bl["ow"]][:, lo:lo + noh, :] += res.reshape(64, noh, U)
            else:
                t = bl["t"]
                # even slot: tile mp (normal), partitions 0:64
                xoff = (mp * NR + r) * B
                lhsT = xt[0:C, xoff:xoff + B]
                res = lhsT.T @ rhs[0:C]
                pts[2 * t][:, lo:lo + noh, :] += res.reshape(64, noh, U)
                # odd slot: tile mp+1 (swapped), partitions 64:128
                xoff = ((mp + 1) * NR + r) * B
                lhsT = xt[C:2 * C, xoff:xoff + B]
                res = lhsT.T @ rhs[C:2 * C]
                pts[2 * t + 1][:, lo:lo + noh, :] += res.reshape(64, noh, U)
        ow = mp - 2
        if 0 <= ow <= OW - 1:
            bv = bias[:, ow * NO * U:(ow + 1) * NO * U].reshape(64, NO, U)
            stag[:, ow] = pts.pop(ow) + bv
    return stag / W_SCALE  # [64, ow, oh_l, u]


# ---------------- device kernel ----------------

def build_nc(loop_n=1):
    """Build the device program. loop_n > 1 wraps the whole phase sweep in a
    hardware For_i loop (identical work each iteration) — used only to
    measure per-iteration HW time above the RPC noise floor."""
    import concourse.bass as bass  # noqa: F401
    import concourse.mybir as mybir
    import concourse.tile as tile
    from concourse import bacc

    phases, totcols, wmax = build_schedule()
    dt = mybir.dt

    ablate = os.environ.get("KABLATE", "")  # dev-only: "nomm","nodve","nodma"
    nc = bacc.Bacc("TRN2", target_bir_lowering=False, debug=False,
                   num_devices=NCORES)
    xt_d = nc.dram_tensor("xt", [128, NMP * NR * B], dt.float16,
                          kind="ExternalInput").ap()
    ws_d = nc.dram_tensor("wstream", [128, totcols], dt.float8e3,
                          kind="ExternalInput").ap()
    bias_d = nc.dram_tensor("bias_1", [1, OW * NO * U], dt.float32,
                            kind="ExternalInput").ap()
    out_d = nc.dram_tensor("out", [B, OW, NO, U], dt.bfloat16,
                           kind="ExternalOutput").ap()

    with tile.TileContext(nc) as tc:
        with tc.tile_pool(name="xpool", bufs=1) as xpool, \
             tc.tile_pool(name="bpool", bufs=1) as bpool, \
             tc.tile_pool(name="stpool", bufs=1) as stpool, \
             tc.tile_pool(name="wpool", bufs=int(os.environ.get("WBUFS","8"))) as wpool, \
             tc.tile_pool(name="tmppool", bufs=4) as tmppool, \
             tc.tile_pool(name="pspool", bufs=int(os.environ.get("PSBUFS","5")), space="PSUM") as pspool:

            # Two HWDGE rings: w phase DMAs alternate between them; the x
            # preload is chunked on the ACT ring so early w phases can start
            # while later x chunks stream in.
            dma_w = nc.sync
            dma_x = nc.scalar

            xsb = xpool.tile([128, NMP * NR * B], dt.float16, tag="xt")
            XCH = int(os.environ.get("XCH", "4"))  # x chunks
            xch = NMP // XCH * NR * B
            for g in range(XCH):
                dma_x.dma_start(xsb[:, g * xch:(g + 1) * xch],
                                xt_d[:, g * xch:(g + 1) * xch])
            b1 = bpool.tile([1, OW * NO * U], dt.float32, tag="b1")
            dma_x.dma_start(b1[:, :], bias_d[:, :])
            bsb = bpool.tile([64, OW * NO * U], dt.float32, tag="brep")
            nc.gpsimd.partition_broadcast(bsb[:, :], b1[:, :], channels=64)
            stag = stpool.tile([64, OW * NO * U], dt.bfloat16)

            import contextlib
            loop_cm = (tc.For_i(0, loop_n, 1) if loop_n > 1
                       else contextlib.nullcontext())
            with loop_cm:
                _emit_sweep(nc, tc, phases, wmax, dt, ablate, dma_w, dma_x,
                            xsb, bsb, stag, ws_d, out_d,
                            wpool, tmppool, pspool)

    nc.compile()
    return nc


def _emit_sweep(nc, tc, phases, wmax, dt, ablate, dma_w, dma_x,
                xsb, bsb, stag, ws_d, out_d, wpool, tmppool, pspool):
    pts = {}
    for mp, blocks in enumerate(phases):
        wcols = sum(bl["ncols"] for bl in blocks)
        if wcols:
            pc0 = blocks[0]["col0"]
            wsb = wpool.tile([128, wmax], dt.float8e3, tag="wstream")
            if ablate != "nodma":
                ring = dma_w if mp % 2 == 0 else dma_x
                ring.dma_start(wsb[:, :wcols],
                               ws_d[:, pc0:pc0 + wcols])

        if mp <= OW - 1:
            pt = pspool.tile([64, NO * U], dt.float32)
            pts[mp] = pt
            if ablate != "nodve":
                nc.vector.memset(pt[:, :], 0.0)

        for bl in blocks:
            r = bl["r"]
            lo, noh = bl["oh0"], bl["noh"]
            loc0 = bl["col0"] - pc0
            if bl["kind"] == "main":
                xoff = (mp * NR + r) * B
                ptv = pts[bl["ow"]][:, :].rearrange(
                    "p (o u) -> p o u", o=NO, u=U)
                lhsT = xsb[:, xoff:xoff + B]
                rhs = wsb[:, loc0:loc0 + bl["ncols"]]
                outap = ptv[:, lo:lo + noh, :]
                if ablate != "nomm":
                    nc.tensor.matmul(outap, lhsT, rhs, start=False,
                                     stop=False, skip_group_check=True)
            else:
                t = bl["t"]
                # even slot: tile mp (normal parity), base 0
                # odd slot:  tile mp+1 (swapped), base 64
                for dh, ow4, tmp_mp in ((0, 2 * t, mp),
                                        (1, 2 * t + 1, mp + 1)):
                    xoff = (tmp_mp * NR + r) * B
                    ptv = pts[ow4][:, :].rearrange(
                        "p (o u) -> p o u", o=NO, u=U)
                    lhsT = xsb[dh * C:(dh + 1) * C, xoff:xoff + B]
                    rhs = wsb[dh * C:(dh + 1) * C,
                              loc0:loc0 + bl["ncols"]]
                    outap = ptv[:, lo:lo + noh, :]
                    if ablate != "nomm":
                        nc.tensor.matmul(outap, lhsT, rhs, start=False,
                                         stop=False,
                                         skip_group_check=True)

        ow = mp - 2
        if 0 <= ow <= OW - 1:
            a1 = pts[ow][:, :].rearrange("p (o u) -> p o u", u=U)
            bv = bsb[:, ow * NO * U:(ow + 1) * NO * U].rearrange(
                "p (o u) -> p o u", u=U)
            stv = stag[:, ow * NO * U:(ow + 1) * NO * U].rearrange(
                "p (o u) -> p o u", u=U)
            if ablate != "nodve":
                nc.vector.tensor_add(stv, a1, bv)
            del pts[ow]
            # stream the output out as rows complete: 8-ow chunks early,
            # then 2-ow chunks so the tail DMAs overlap the final drains
            if ow < 24 and ow % 8 == 7:
                g = ow // 8
                sl = slice(g * 8 * NO * U, (g + 1) * 8 * NO * U)
                dma_w.dma_start(
                    out_d.rearrange("b w o u -> b (w o u)")[:, sl],
                    stag[:, sl])
            elif ow >= 24 and ow % 2 == 1:
                sl = slice((ow - 1) * NO * U, (ow + 1) * NO * U)
                dma_w.dma_start(
                    out_d.rearrange("b w o u -> b (w o u)")[:, sl],
                    stag[:, sl])


def _exec(nc, in_maps, repeats=1, chain=1):
    """Execute the prebuilt Bass module on the 8 cores via PJRT/axon.

    Mirrors bass2jax.run_bass_via_pjrt's multi-core branch, but keeps the
    jitted executable + device-staged inputs so the kernel can be re-run for
    timing. `chain` repeats the kernel execution inside one program (for
    amortized on-device timing). Returns (per_core_results, wall_times_s).
    """
    import time

    import jax
    import numpy as _np
    from jax.sharding import Mesh, NamedSharding, PartitionSpec

    try:
        from jax.experimental.shard_map import shard_map
    except ImportError:
        from jax.shard_map import shard_map

    import concourse.mybir as mybir
    from concourse import bass2jax

    bass2jax.install_neuronx_cc_hook()

    partition_name = (nc.partition_id_tensor.name
                      if nc.partition_id_tensor else None)
    in_names, out_names, out_avals, zero_outs = [], [], [], []
    for alloc in nc.m.functions[0].allocations:
        if not isinstance(alloc, mybir.MemoryLocationSet):
            continue
        name = alloc.memorylocations[0].name
        if alloc.kind == "ExternalInput":
            if name != partition_name:
                in_names.append(name)
        elif alloc.kind == "ExternalOutput":
            out_names.append(name)
            shape = tuple(alloc.tensor_shape)
            dtype = mybir.dt.np(alloc.dtype)
            out_avals.append(jax.core.ShapedArray(shape, dtype))
            zero_outs.append(_np.zeros(shape, dtype))
    n_params = len(in_names)
    all_names = in_names + out_names
    if partition_name is not None:
        all_names = all_names + [partition_name]

    def _bind(operands):
        return bass2jax._bass_exec_p.bind(
            *operands,
            out_avals=tuple(out_avals),
            in_names=tuple(all_names),
            out_names=tuple(out_names),
            lowering_input_output_aliases=(),
            sim_require_finite=True,
            sim_require_nnan=True,
            nc=nc,
        )

    def _body(*args):
        operands = list(args)
        if partition_name is not None:
            operands.append(bass2jax.partition_id_tensor())
        return tuple(_bind(operands))

    n_cores = len(in_maps)
    devices = jax.devices()[:n_cores]
    mesh = Mesh(_np.asarray(devices), ("core",))
    spec = PartitionSpec("core")
    sharded = jax.jit(
        shard_map(_body, mesh=mesh, in_specs=(spec,) * (n_params + len(out_names)),
                  out_specs=(spec,) * len(out_names), check_rep=False),
        keep_unused=True,
    )
    sharding = NamedSharding(mesh, spec)
    staged = [
        jax.device_put(
            _np.concatenate([_np.asarray(m[name]) for m in in_maps], axis=0),
            sharding)
        for name in in_names
    ] + [
        jax.device_put(
            _np.zeros((n_cores * z.shape[0], *z.shape[1:]), z.dtype), sharding)
        for z in zero_outs
    ]

    times = []
    out_arrs = None
    for _ in range(max(1, repeats)):
        t0 = time.perf_counter()
        out_arrs = jax.block_until_ready(sharded(*staged))
        times.append(time.perf_counter() - t0)

    results = [
        {
            name: _np.asarray(out_arrs[i]).reshape(n_cores, *out_avals[i].shape)[c]
            for i, name in enumerate(out_names)
        }
        for c in range(n_cores)
    ]
    return results, times


def _run(inputs, repeats=1):
    """Run on hardware. Returns (full_output, wall_times_s)."""
    in_maps = pack_inputs(inputs["x"], inputs["w"], inputs["b"])
    nc = build_nc()
    results, times = _exec(nc, in_maps, repeats=repeats)
    return _gather(results), times


def _gather(results):
    out = np.empty((B, OHP, OW, U), dtype=np.float32)
    for c in range(NCORES):
        # per-core out [B, OW, NO, U] bf16 (scaled by W_SCALE)
        out[:, c * NO:(c + 1) * NO] = (
            results[c]["out"].astype(np.float32).transpose(0, 2, 1, 3))
    return out[:, :OH] * (1.0 / W_SCALE)


def kernel(x, w, b):
    from concourse.bass_utils import run_bass_kernel_spmd

    in_maps = pack_inputs(x, w, b)
    nc = build_nc()
    res = run_bass_kernel_spmd(nc, in_maps, list(range(NCORES)))
    return _gather(res.results)

